# revision 1
# baseline (speedup 1.0000x reference)
"""CoPEGate Trainium2 kernel.

Computes out[b,h,t,s] = sigmoid((Q K^T)[b,h,t,s] / sqrt(D)) * (P P^T)[t,s] / sqrt(D)
for B=2, H=12, T=2048, D=64 (fp32), distributed over 8 NeuronCores.

Sharding: the 24 (b,h) pairs are split 3-per-core (head-parallel); the
positional matrix P is replicated and its T x T bias is computed on every
core (it is reused across that core's 3 heads). No cross-device
communication is needed.

Per-core dataflow (all shapes per core):
  inputs   qT, kT: [3, 64, 2048] fp16 (host pre-transposed so the matmul
           contraction dim D=64 lands on SBUF partitions, and host-cast to
           fp16 for full-rate 2-byte matmuls), pT: [64, 2048] fp32 (f32r
           matmul, so the pos bias factor stays near-fp32).
  loop over 16 row-tiles of 128:
    pos stripe   = PE matmul pT[:, tile].T @ pT          -> PSUM [128, 2048]
                   ScalarE Copy * 1/sqrt(D)              -> SBUF
    per head h:  scores = PE matmul qT[h][:, tile].T @ kT[h] -> PSUM
                 gate   = ScalarE Sigmoid(scores / sqrt(D))  -> SBUF
                 out    = VectorE gate * pos stripe          -> SBUF
                 one 1 MiB contiguous DMA per stripe         -> HBM
The kernel is output-DMA bound (~50 MiB written per core; core pairs
share a 716 GB/s HBM stack, so the per-core floor is ~147 us of wire
time). Measured 158-175 us per core end-to-end, varying with HBM
write-arbitration luck against the paired core.

Ramp-up: inputs are DMA'd as 512-wide column chunks (separate tiles =>
separate scheduler dependencies) and row-tile 0 is emitted chunk-major
in dataflow order, so the first output bytes reach HBM ~12 us after
kernel start instead of ~25.

PE utilization: the K=64 contraction only uses half the 128-row PE
array, so operands are laid out in alternating partition halves --
pos & head0 at partitions 0-63, heads 1 & 2 at partitions 64-127 -- and
stripes are issued in the order pos, h1, h0, h2. Adjacent stripes'
matmuls target disjoint PE row groups (bass auto-derives tile_position
from the operand base partition) and execute concurrently in the array.

Precision: scores matmuls take host-rounded fp16 q/k (10 mantissa bits,
8x less rounding error than bf16 at the same wire/PE cost); the pos
bias matmul runs in float32r (TF32-like, full fp32 rate). End-to-end:
rel err 1.8e-4, absmax 4.1e-3 -- within ~20% of the all-f32r variant's
accuracy while keeping the bf16 variant's speed.
"""

import math
import os
import sys

import numpy as np

sys.path.insert(0, "/opt/trn_rl_repo")

B, H, T, D = 2, 12, 2048, 64
N_CORES = 8
HPC = (B * H) // N_CORES  # heads per core
PT = 128  # output row-tile height (SBUF/PSUM partitions)
NT = T // PT  # row tiles
NCHUNK = 512  # matmul moving-operand free dim (one PSUM bank of fp32)
NCH = T // NCHUNK
INV_SQRT_D = 1.0 / math.sqrt(D)

_NC_CACHE = {}


def _build_nc():
    import concourse.bass as bass
    from concourse import bacc, mybir, tile

    f32 = mybir.dt.float32
    f32r = mybir.dt.float32r
    f16 = mybir.dt.float16
    Sigmoid = mybir.ActivationFunctionType.Sigmoid
    Copy = mybir.ActivationFunctionType.Copy

    nc = bacc.Bacc("TRN2", target_bir_lowering=False)

    qT = nc.dram_tensor("qT", [HPC, D, T], f16, kind="ExternalInput")
    kT = nc.dram_tensor("kT", [HPC, D, T], f16, kind="ExternalInput")
    pT = nc.dram_tensor("pT", [D, T], f32r, kind="ExternalInput")
    out = nc.dram_tensor("out", [HPC, T, T], f32, kind="ExternalOutput")

    with tile.TileContext(nc) as tc:
        with tc.tile_pool(name="ins", bufs=1) as ins_pool, \
             tc.tile_pool(name="pos", bufs=2) as pos_pool, \
             tc.tile_pool(name="gate", bufs=6) as gate_pool, \
             tc.tile_pool(name="outs", bufs=12) as outs_pool, \
             tc.tile_pool(name="ps", bufs=2, space="PSUM") as ps_pool:

            # Inputs live as 512-wide column-chunk tiles so the first
            # matmul only waits for ~0.3 MiB of input DMA, not all 2.6 MiB
            # (separate tiles => separate dependencies for the scheduler).
            # Heads 0+1 share [128, 512] tiles (contiguous full-port DMA,
            # head 1 on partitions 64-127); head 2 occupies the upper half
            # of its own tiles so its matmuls use the upper PE row group.
            k01_c, q01_c, p_c, k2_c, q2_c = [], [], [], [], []
            qT01 = qT[0:2].rearrange("h d t -> (h d) t")
            kT01 = kT[0:2].rearrange("h d t -> (h d) t")
            for j in range(NCH):
                jsl = bass.ts(j, NCHUNK)
                kc = ins_pool.tile([2 * D, NCHUNK], f16, tag=f"k01_{j}")
                nc.sync.dma_start(out=kc, in_=kT01[:, jsl])
                k01_c.append(kc)
                qc = ins_pool.tile([2 * D, NCHUNK], f16, tag=f"q01_{j}")
                nc.sync.dma_start(out=qc, in_=qT01[:, jsl])
                q01_c.append(qc)
                pc = ins_pool.tile([D, NCHUNK], f32r, tag=f"p_{j}")
                nc.sync.dma_start(out=pc, in_=pT[:, jsl])
                p_c.append(pc)
            for j in range(NCH):
                jsl = bass.ts(j, NCHUNK)
                kc = ins_pool.tile([2 * D, NCHUNK], f16, tag=f"k2_{j}")
                nc.sync.dma_start(out=kc[D : 2 * D, :], in_=kT[2][:, jsl])
                k2_c.append(kc)
                qc = ins_pool.tile([2 * D, NCHUNK], f16, tag=f"q2_{j}")
                nc.sync.dma_start(out=qc[D : 2 * D, :], in_=qT[2][:, jsl])
                q2_c.append(qc)

            def q_lhsT(h, it):
                # lhsT [64, 128] = q chunk tile (it//4), 128-col slice.
                sl = bass.ts(it % (NCHUNK // PT), PT)
                if h == 2:
                    return q2_c[it // (NCHUNK // PT)][D : 2 * D, sl]
                lo, hi = (0, D) if h == 0 else (D, 2 * D)
                return q01_c[it // (NCHUNK // PT)][lo:hi, sl]

            def p_lhsT(it):
                sl = bass.ts(it % (NCHUNK // PT), PT)
                return p_c[it // (NCHUNK // PT)][:, sl]

            def rhs(h, j):
                if h is None:
                    return p_c[j][:, :]
                if h == 2:
                    return k2_c[j][D : 2 * D, :]
                lo, hi = (0, D) if h == 0 else (D, 2 * D)
                return k01_c[j][lo:hi, :]

            def mm_stripe(psum, h, it):
                for j in range(NCH):
                    nc.tensor.matmul(
                        psum[:, bass.ts(j, NCHUNK)],
                        p_lhsT(it) if h is None else q_lhsT(h, it),
                        rhs(h, j),
                        start=True,
                        stop=True,
                    )

            def post(h, sp, pos_sb, tsl, npost=1):
                cw = T // npost
                gate = gate_pool.tile([PT, T], f32, tag="gate")
                o = outs_pool.tile([PT, T], f32, tag="o")
                for c in range(npost):
                    csl = bass.ts(c, cw)
                    nc.scalar.activation(
                        gate[:, csl], sp[:, csl], Sigmoid, scale=INV_SQRT_D
                    )
                    nc.vector.tensor_mul(o[:, csl], gate[:, csl], pos_sb[:, csl])
                    nc.sync.dma_start(out=out[h, tsl, csl], in_=o[:, csl])

            # --- tile 0: chunk-major software pipeline -----------------
            # Emitted in dataflow order, chunk by chunk, so every engine's
            # FIFO sees tile 0's chunk c before chunk c+1 work and the
            # first output bytes reach HBM as early as possible. Head 1
            # leads because its q/k chunks are DMA'd first.
            tsl0 = bass.ts(0, PT)
            sp1 = ps_pool.tile([PT, T], f32, tag="ps")
            pp0 = ps_pool.tile([PT, T], f32, tag="ps")
            pos0 = pos_pool.tile([PT, T], f32, tag="pos")
            gate0 = gate_pool.tile([PT, T], f32, tag="gate")
            o0 = outs_pool.tile([PT, T], f32, tag="o")
            for c in range(NCH):
                csl = bass.ts(c, NCHUNK)
                nc.tensor.matmul(
                    sp1[:, csl], q_lhsT(1, 0), rhs(1, c), start=True, stop=True
                )
                nc.tensor.matmul(
                    pp0[:, csl], p_lhsT(0), rhs(None, c), start=True, stop=True
                )
                nc.scalar.activation(
                    gate0[:, csl], sp1[:, csl], Sigmoid, scale=INV_SQRT_D
                )
                nc.scalar.activation(
                    pos0[:, csl], pp0[:, csl], Copy, scale=INV_SQRT_D
                )
                nc.vector.tensor_mul(o0[:, csl], gate0[:, csl], pos0[:, csl])
                nc.sync.dma_start(out=out[1, tsl0, csl], in_=o0[:, csl])
            for h in (0, 2):
                sp = ps_pool.tile([PT, T], f32, tag="ps")
                mm_stripe(sp, h, 0)
                post(h, sp, pos0, tsl0)

            # --- steady-state tiles ------------------------------------
            # Stripe order alternates PE row groups (pos/h0 on array rows
            # 0-63, h1/h2 on 64-127) so adjacent stripes' matmuls overlap
            # in the PE array.
            for it in range(1, NT):
                tsl = bass.ts(it, PT)
                pp = ps_pool.tile([PT, T], f32, tag="ps")
                mm_stripe(pp, None, it)
                pos_sb = pos_pool.tile([PT, T], f32, tag="pos")
                nc.scalar.activation(pos_sb, pp, Copy, scale=INV_SQRT_D)
                for h in (1, 0, 2):
                    sp = ps_pool.tile([PT, T], f32, tag="ps")
                    mm_stripe(sp, h, it)
                    post(h, sp, pos_sb, tsl)

    nc.finalize()
    return nc


def _get_nc():
    if "nc" not in _NC_CACHE:
        _NC_CACHE["nc"] = _build_nc()
    return _NC_CACHE["nc"]


def kernel(query, key, pos_embed_weight):
    query = np.asarray(query, dtype=np.float32)
    key = np.asarray(key, dtype=np.float32)
    pos_embed_weight = np.asarray(pos_embed_weight, dtype=np.float32)

    q = query.reshape(B * H, T, D)
    k = key.reshape(B * H, T, D)
    p_t = np.ascontiguousarray(pos_embed_weight[:T].T)  # [D, T]

    in_maps = []
    for c in range(N_CORES):
        hs = slice(c * HPC, (c + 1) * HPC)
        in_maps.append(
            {
                "qT": np.ascontiguousarray(
                    q[hs].transpose(0, 2, 1).astype(np.float16)
                ),
                "kT": np.ascontiguousarray(
                    k[hs].transpose(0, 2, 1).astype(np.float16)
                ),
                "pT": p_t,
            }
        )

    from concourse.bass_utils import run_bass_kernel_spmd

    nc = _get_nc()
    try:
        res = run_bass_kernel_spmd(
            nc,
            in_maps,
            core_ids=list(range(N_CORES)),
            trace=bool(os.environ.get("KERNEL_TRACE")),
        )
    except Exception:
        # One retry for transient runtime/compile hiccups.
        res = run_bass_kernel_spmd(
            nc, in_maps, core_ids=list(range(N_CORES)), trace=False
        )
    kernel.last_results = res

    full = np.empty((B * H, T, T), dtype=np.float32)
    for c in range(N_CORES):
        full[c * HPC : (c + 1) * HPC] = res.results[c]["out"]
    return full.reshape(B, H, T, T)


kernel.last_results = None

